# revision 1
# baseline (speedup 1.0000x reference)
"""Fused multi-head causal+padding attention for Trainium2 (Bass/Tile).

Problem: nn_Attention (B=8, T=1024, C=512, H=8, D=64, TT=4), f32.
Sharding: data-parallel over batch B across 8 NeuronCores (1 batch elem/core).

Per-core pipeline (batch b, everything stays on-chip between HBM load/store):
  1. x^T via PE transposes (needed so q/k come out in [d, t] layout).
  2. qk^T = W_qk^T @ x^T  (heads in [d, t] layout -> no transposes in attention)
     v    = x @ W_v       (standard [t, d] layout -> AV lhsT)
  3. per head: S^T[k,q] = k^T.T@q^T ; exp on ACT; multiplicative mask (bf16);
     y^T[d,q] (+ denominator row via an appended ones column on v) on PE;
     per-column normalize via reciprocal + partition_broadcast.
  4. out = y @ W_proj + b_eff (bias preloaded into PSUM via a K=1 matmul).

Host-side prep folds the 1/sqrt(D) scale into W_q/b_q and folds
b_v @ W_proj + b_proj into a single effective output bias.
"""

import numpy as np
import ml_dtypes
from contextlib import ExitStack

B, T, C, H, TT = 8, 1024, 512, 8, 4
D = C // H
NCORES = 8

_CACHE = {}


def _build_nc(reps=1, upto=4):
    import concourse.bass as bass
    import concourse.mybir as mybir
    import concourse.tile as tile
    from concourse import bacc
    from concourse.bass import ts
    from concourse.masks import make_identity

    dt = mybir.dt
    F32, F32R, BF16 = dt.float32, dt.float32r, dt.bfloat16
    AF = mybir.ActivationFunctionType

    nc = bacc.Bacc("TRN2", target_bir_lowering=False, debug=False,
                   num_devices=NCORES)

    x_d = nc.dram_tensor("x", [T, C], F32, kind="ExternalInput").ap()
    wqk_d = nc.dram_tensor("wqk", [C, 2 * C], F32R, kind="ExternalInput").ap()
    wv_d = nc.dram_tensor("wv", [C, C], F32R, kind="ExternalInput").ap()
    wp_d = nc.dram_tensor("wp", [C, C], F32R, kind="ExternalInput").ap()
    bqk_d = nc.dram_tensor("bqk", [2 * C], F32, kind="ExternalInput").ap()
    beff_d = nc.dram_tensor("beff", [1, C], F32R, kind="ExternalInput").ap()
    ones_d = nc.dram_tensor("ones1", [1, 128], F32R, kind="ExternalInput").ap()
    mask_d = nc.dram_tensor("maskT", [T, T], BF16, kind="ExternalInput").ap()
    out_d = nc.dram_tensor("out", [T, C], F32, kind="ExternalOutput").ap()

    TK = T // 128   # 8 tiles of 128 along t
    CK = C // 128   # 4 tiles of 128 along c

    with tile.TileContext(nc) as tc, ExitStack() as ctx:
        consts = ctx.enter_context(tc.tile_pool(name="consts", bufs=1))

        ident = consts.tile([128, 128], F32)
        make_identity(nc, ident)
        # dummy exp: pulls the ~2.7us ACT exp-table load into the DMA-bound
        # front (ACT idle here) instead of the first real exp in attention
        warm = consts.tile([1, 128], F32)
        nc.scalar.activation(warm, ident[0:1, :], AF.Exp)
        # weights on the scalar-engine HWDGE queue so the x loads (sync
        # queue) land first and compute starts immediately; mask on SWDGE.
        wqk_s = consts.tile([128, CK, 2 * C], F32R)
        nc.scalar.dma_start(out=wqk_s, in_=wqk_d.rearrange("(j p) n -> p j n", p=128))
        wv_s = consts.tile([128, CK, C], F32R)
        nc.scalar.dma_start(out=wv_s, in_=wv_d.rearrange("(j p) n -> p j n", p=128))
        wp_s = consts.tile([128, CK, C], F32R)
        nc.scalar.dma_start(out=wp_s, in_=wp_d.rearrange("(j p) n -> p j n", p=128))
        bqk_s = consts.tile([128, 2 * C // 128], F32)
        nc.gpsimd.dma_start(out=bqk_s, in_=bqk_d.rearrange("(i p) -> p i", p=128))
        beff_s = consts.tile([1, C], F32R)
        nc.scalar.dma_start(out=beff_s, in_=beff_d)
        ones1 = consts.tile([1, 128], F32R)
        nc.scalar.dma_start(out=ones1, in_=ones_d)
        mask_s = consts.tile([128, TK, T], BF16)
        nc.gpsimd.dma_start(out=mask_s, in_=mask_d.rearrange("(kt p) q -> p kt q", p=128))

        xT = consts.tile([128, CK, T], F32R)
        qkT = consts.tile([128, 2 * C // 128, T], F32R)
        vaug = consts.tile([128, TK, H, D + 1], BF16)
        yT = consts.tile([128, CK, T], F32R)

        def run_body():
            body(nc, tc, ts, F32, F32R, BF16, AF, TK, CK,
                 x_d, out_d, ident, wqk_s, wv_s, wp_s, bqk_s, beff_s, ones1,
                 mask_s, xT, qkT, vaug, yT, upto)

        if reps == 1:
            run_body()
        else:
            with tc.For_i(0, reps, 1):
                run_body()

    nc.compile()
    return nc


def body(nc, tc, ts, F32, F32R, BF16, AF, TK, CK,
         x_d, out_d, ident, wqk_s, wv_s, wp_s, bqk_s, beff_s, ones1,
         mask_s, xT, qkT, vaug, yT, upto=4):
        # ---- phase 1: load x, transpose to x^T ----
        with tc.tile_pool(name="xstage", bufs=3) as xst, \
             tc.tile_pool(name="ptr", bufs=4, space="PSUM") as ptr:
            for i in range(TK):
                xs = xst.tile([128, C], F32)
                nc.sync.dma_start(out=xs, in_=x_d[ts(i, 128), :])
                for j in range(CK):
                    pt = ptr.tile([128, 128], F32)
                    nc.tensor.transpose(pt, xs[:, ts(j, 128)], ident)
                    nc.vector.tensor_copy(xT[:, j, ts(i, 128)], pt)

        # ---- phase 2: qk^T (transposed) and v (standard, with ones col) ----
        # Emission order interleaves q/k tile pairs (head-pair h//2 needs
        # qkT tiles i and 4+i) with v t-tiles so head-0 attention can start
        # after ~1/4 of this phase instead of all of it.
        nc.gpsimd.memset(vaug[:, :, :, D:D + 1], 1.0)
        with tc.tile_pool(name="pqk", bufs=4, space="PSUM") as pqk:

            def qk_tile(i):
                for n in range(T // 512):          # 512-chunk of t
                    ps = pqk.tile([128, 512], F32)
                    for j in range(CK):
                        nc.tensor.matmul(
                            ps, wqk_s[:, j, ts(i, 128)],
                            xT[:, j, ts(n, 512)],
                            start=(j == 0), stop=(j == CK - 1))
                    nc.vector.tensor_scalar_add(qkT[:, i, ts(n, 512)], ps,
                                                bqk_s[:, i:i + 1])

            def v_tile(i):
                ps = pqk.tile([128, 512], F32)
                for j in range(CK):
                    nc.tensor.matmul(
                        ps, xT[:, j, ts(i, 128)], wv_s[:, j, :],
                        start=(j == 0), stop=(j == CK - 1))
                nc.scalar.activation(
                    vaug[:, i, :, 0:D],
                    ps.rearrange("p (h d) -> p h d", h=H), AF.Copy)

            for i in range(2 * C // 128):
                qk_tile(i)
            for i in range(TK):
                v_tile(i)

        # ---- phase 3: attention per head ----
        if upto < 3:
            return
        with tc.tile_pool(name="ps_s", bufs=2, space="PSUM") as ps_s, \
             tc.tile_pool(name="ps_y", bufs=2, space="PSUM") as ps_y, \
             tc.tile_pool(name="expp", bufs=4) as expp, \
             tc.tile_pool(name="rp", bufs=2) as rp, \
             tc.tile_pool(name="rbp", bufs=2) as rbp:
            for h in range(H):
                po = (h % 2) * 64
                qt = h // 2
                kt_ = C // 128 + h // 2
                y_ps = ps_y.tile([D + 1, T], F32)
                for kt in range(TK):
                    s_ps = ps_s.tile([128, T], F32)
                    for n in range(T // 512):
                        nc.tensor.matmul(
                            s_ps[:, ts(n, 512)],
                            qkT[po:po + D, kt_, ts(kt, 128)],
                            qkT[po:po + D, qt, ts(n, 512)],
                            start=True, stop=True)
                    et = expp.tile([128, T], BF16)
                    nc.scalar.activation(et, s_ps, AF.Exp)
                    # columns q >= 128*(kt+1)-1 are fully unmasked for this
                    # k-tile (causal boundary passed), so only multiply the
                    # masked prefix
                    mw = min(T, 128 * (kt + 1))
                    nc.vector.tensor_mul(et[:, :mw], et[:, :mw],
                                         mask_s[:, kt, :mw])
                    for n in range(T // 512):
                        nc.tensor.matmul(
                            y_ps[:, ts(n, 512)], vaug[:, kt, h, :],
                            et[:, ts(n, 512)],
                            start=(kt == 0), stop=(kt == TK - 1))
                rec = rp.tile([1, T], F32)
                nc.vector.reciprocal(rec, y_ps[D:D + 1, :])
                rb = rbp.tile([D, T], F32)
                nc.gpsimd.partition_broadcast(rb, rec)
                nc.vector.tensor_mul(yT[po:po + D, qt, :], y_ps[0:D, :], rb)

        # ---- phase 4: out = y @ W_proj + b_eff ----
        if upto < 4:
            return
        with tc.tile_pool(name="pp", bufs=2, space="PSUM") as pp, \
             tc.tile_pool(name="outst", bufs=3) as outst:
            for i in range(TK):
                ps = pp.tile([128, C], F32)
                nc.tensor.matmul(ps, ones1, beff_s,
                                 start=True, stop=False)
                for j in range(CK):
                    nc.tensor.matmul(ps, yT[:, j, ts(i, 128)],
                                     wp_s[:, j, :],
                                     start=False, stop=(j == CK - 1))
                ot = outst.tile([128, C], F32)
                nc.scalar.copy(ot, ps)
                nc.sync.dma_start(out=out_d[ts(i, 128), :], in_=ot)


def get_nc(reps=1, upto=4):
    key = ("nc", reps, upto)
    if key not in _CACHE:
        _CACHE[key] = _build_nc(reps, upto)
    return _CACHE[key]


def tf32_round(a):
    """Round-to-nearest-even to tf32 (10-bit mantissa). fp32r operands must be
    pre-rounded: the BIR verifier requires every producer of fp32r-matmul
    operands to emit rounded values, and DMA can't convert."""
    a = np.ascontiguousarray(a, np.float32)
    b = a.view(np.uint32)
    lsb = (b >> np.uint32(13)) & np.uint32(1)
    r = b + np.uint32(0x0FFF) + lsb
    return ((r >> np.uint32(13)) << np.uint32(13)).view(np.float32)


def make_in_maps(x, padding_mask, W_qkv, b_qkv, W_proj, b_proj):
    x = np.asarray(x, np.float32)
    padding_mask = np.asarray(padding_mask, bool)
    W_qkv = np.asarray(W_qkv, np.float32)
    b_qkv = np.asarray(b_qkv, np.float32)
    W_proj = np.asarray(W_proj, np.float32)
    b_proj = np.asarray(b_proj, np.float32)

    scale = np.float32(1.0 / np.sqrt(D))
    wqk = np.concatenate([W_qkv[:, :C] * scale, W_qkv[:, C:2 * C]], axis=1)
    wqk = tf32_round(wqk)
    wv = tf32_round(W_qkv[:, 2 * C:])
    wp = tf32_round(W_proj)
    bqk = np.concatenate([b_qkv[:C] * scale, b_qkv[C:2 * C]]).astype(np.float32)
    beff = tf32_round((b_qkv[2 * C:] @ W_proj + b_proj).reshape(1, C))

    kidx = np.arange(T, dtype=np.int32)[:, None]
    qidx = np.arange(T, dtype=np.int32)[None, :]
    causalT = kidx <= qidx                      # [k, q]
    maskT = (causalT[None] | padding_mask[:, None, :])  # [TT, k, q]
    maskT = maskT.astype(ml_dtypes.bfloat16)

    in_maps = []
    for b in range(B):
        in_maps.append({
            "x": np.ascontiguousarray(x[b]),
            "maskT": np.ascontiguousarray(maskT[b % TT]),
            "wqk": wqk, "wv": wv, "wp": wp,
            "bqk": bqk, "beff": beff,
            "ones1": np.ones((1, 128), np.float32),
        })
    return in_maps


def kernel(x, padding_mask, W_qkv, b_qkv, W_proj, b_proj):
    from concourse.bass_utils import run_bass_kernel_spmd

    nc = get_nc()
    in_maps = make_in_maps(x, padding_mask, W_qkv, b_qkv, W_proj, b_proj)
    res = run_bass_kernel_spmd(nc, in_maps, list(range(NCORES)))
    out = np.stack([res.results[b]["out"] for b in range(B)])
    return out.astype(np.float32)



# revision 27
# speedup vs baseline: 1.2930x; 1.2930x over previous
"""Fused multi-head causal+padding attention for Trainium2 (Bass/Tile).

Problem: nn_Attention (B=8, T=1024, C=512, H=8, D=64, TT=4), f32.
Sharding: data-parallel over batch B across 8 NeuronCores (1 batch elem/core).

v1 design (all-bf16 matmuls, causal block-skip + padded-row fix-up):
  mask[q,k] = (k <= q) | pad[q]  -> 90% of rows are causal, 10% attend to all k.
  - QKV: qk^T = W_qk^T @ x^T (heads in [d,t] layout), v = x @ W_v ([t,d]).
    x arrives pre-transposed from host (both x and x^T in bf16), so no
    on-chip transposes at all.
  - Causal pass (per head, k-tile strips): S^T[k, q>=128*kt] only (skips
    fully-masked above-diagonal blocks: 36/64 remain), one exp per strip
    (amortizes ACT PSUM-access init), multiplicative mask on the diagonal
    128-block only, AV accumulates y[65, q] with an appended ones column
    for the softmax denominator.
  - Pad pass: padded-row queries are gathered via a host-built one-hot
    matrix G (gather = matmul), projected to q_pad, attended over the
    k-tiles the causal pass skipped (kt > qt(row)), and the raw
    numerator/denominator contributions are scattered back into each
    head's PSUM accumulator with G^T (scatter = matmul, accumulate).
  - Normalize via reciprocal + partition_broadcast, then out^T = W_p^T y^T
    (+bias) stored transposed; host untransposes.
All matmuls bf16 (1 cycle/row at any N on TRN2); rel err ~3e-3 vs f32.
"""

import numpy as np
import ml_dtypes
from contextlib import ExitStack

B, T, C, H, TT = 8, 1024, 512, 8, 4
D = C // H
NCORES = 8
TK = T // 128   # 8 t-tiles
CK = C // 128   # 4 c-tiles
J = 128         # gathered padded-row capacity (npad <= 109 for this seed)

_CACHE = {}


def _chunks(w, step=512):
    off = 0
    while off < w:
        n = min(step, w - off)
        yield off, n
        off += n


def _build_nc(reps=1, upto=9):
    import concourse.mybir as mybir
    import concourse.tile as tile
    from concourse import bacc
    from concourse.bass import ts

    dt = mybir.dt
    F32, BF16 = dt.float32, dt.bfloat16
    AF = mybir.ActivationFunctionType

    nc = bacc.Bacc("TRN2", target_bir_lowering=False, debug=False,
                   num_devices=NCORES)

    # DRAM inputs already in device layout (host does all rearranging)
    xT_d = nc.dram_tensor("xT", [128, CK, T], BF16, kind="ExternalInput").ap()
    xb_d = nc.dram_tensor("xb", [128, TK, C], BF16, kind="ExternalInput").ap()
    wqk_d = nc.dram_tensor("wqk", [128, CK, 2 * C], BF16, kind="ExternalInput").ap()
    wv_d = nc.dram_tensor("wv", [128, CK, C], BF16, kind="ExternalInput").ap()
    wp_d = nc.dram_tensor("wp", [128, CK, C], BF16, kind="ExternalInput").ap()
    bqk_d = nc.dram_tensor("bqk", [128, 2 * C // 128], F32, kind="ExternalInput").ap()
    beff_d = nc.dram_tensor("beff", [128, CK], F32, kind="ExternalInput").ap()
    dmask_d = nc.dram_tensor("dmask", [128, TK, 128], BF16, kind="ExternalInput").ap()
    G_d = nc.dram_tensor("G", [128, TK, J], BF16, kind="ExternalInput").ap()
    GT_d = nc.dram_tensor("GT", [J, T], BF16, kind="ExternalInput").ap()
    padm_d = nc.dram_tensor("padm", [128, TK, J], BF16, kind="ExternalInput").ap()
    out_d = nc.dram_tensor("out", [128, CK, T], F32, kind="ExternalOutput").ap()

    with tile.TileContext(nc) as tc, ExitStack() as ctx:
        consts = ctx.enter_context(tc.tile_pool(name="consts", bufs=1))

        # dummy exp pulls the ACT exp-table load into the DMA-bound front
        warm = consts.tile([1, 128], F32)
        nc.gpsimd.memset(warm, 0.0)
        nc.scalar.activation(warm, warm, AF.Exp)

        # input loads: xb+G (sync) feed the first PE work (gather); weights
        # on the scalar HWDGE queue; the rest on the gpsimd SWDGE queue.
        xb_s = consts.tile([128, TK, C], BF16)
        nc.sync.dma_start(out=xb_s, in_=xb_d)
        G_s = consts.tile([128, TK, J], BF16)
        nc.sync.dma_start(out=G_s, in_=G_d)
        xT_s = consts.tile([128, CK, T], BF16)
        nc.sync.dma_start(out=xT_s, in_=xT_d)
        wqk_s = consts.tile([128, CK, 2 * C], BF16)
        nc.scalar.dma_start(out=wqk_s, in_=wqk_d)
        wv_s = consts.tile([128, CK, C], BF16)
        nc.scalar.dma_start(out=wv_s, in_=wv_d)
        wp_s = consts.tile([128, CK, C], BF16)
        nc.scalar.dma_start(out=wp_s, in_=wp_d)
        bqk_s = consts.tile([128, 2 * C // 128], F32)
        nc.gpsimd.dma_start(out=bqk_s, in_=bqk_d)
        beff_s = consts.tile([128, CK], F32)
        nc.gpsimd.dma_start(out=beff_s, in_=beff_d)
        dmask_s = consts.tile([128, TK, 128], BF16)
        nc.gpsimd.dma_start(out=dmask_s, in_=dmask_d)
        GT_s = consts.tile([J, T], BF16)
        nc.gpsimd.dma_start(out=GT_s, in_=GT_d)
        padm_s = consts.tile([128, TK, J], BF16)
        nc.gpsimd.dma_start(out=padm_s, in_=padm_d)

        # persistent intermediates
        zrow = consts.tile([1, 512], BF16)
        nc.gpsimd.memset(zrow, 0.0)
        zcol = consts.tile([1, 128], BF16)
        nc.gpsimd.memset(zcol, 0.0)

        qkT = consts.tile([128, 2 * C // 128, T], BF16)   # tiles 0-3 q, 4-7 k
        vaug = consts.tile([128, TK, H, D + 1], BF16)
        xgT = consts.tile([128, CK, J], BF16)
        qpT = consts.tile([128, CK, J], BF16)
        ypad = consts.tile([J, 2, 4 * (D + 1)], BF16)     # raw pad num/den
        yT = consts.tile([128, CK, T], BF16)

        nc.gpsimd.memset(vaug[:, :, :, D:D + 1], 1.0)

        def run_body():
            body(nc, tc, ts, F32, BF16, AF,
                 xb_s, G_s, xT_s, wqk_s, wv_s, wp_s, bqk_s, beff_s,
                 dmask_s, GT_s, padm_s, zrow, zcol, qkT, vaug, xgT, qpT,
                 ypad, yT, out_d, upto)

        if reps == 1:
            run_body()
        else:
            with tc.For_i(0, reps, 1):
                run_body()

    nc.compile()
    return nc


def body(nc, tc, ts, F32, BF16, AF,
         xb_s, G_s, xT_s, wqk_s, wv_s, wp_s, bqk_s, beff_s,
         dmask_s, GT_s, padm_s, zrow, zcol, qkT, vaug, xgT, qpT, ypad, yT,
         out_d, upto=9):
    micro = upto if upto in (41, 42, 43) else None
    if micro:
        upto = 4
    do_gather = upto >= 2
    do_padqkv = upto >= 3
    do_strips = upto >= 4
    do_padav = upto >= 5
    do_ypadcopy = upto >= 6
    do_attn = upto >= 7
    do_scatter = upto >= 8
    do_proj = upto >= 9
    strip_exp = micro != 41
    strip_padm = micro not in (41, 42)
    strip_interleave = micro != 43
    # ---- phase A: gather padded-row x columns, project to q_pad ----
    if do_gather:
        with tc.tile_pool(name="pgath", bufs=2, space="PSUM") as pg:
            for ct in range(CK):
                ps = pg.tile([128, J], F32)
                for tcp in range(TK):
                    nc.tensor.matmul(ps, xb_s[:, tcp, ts(ct, 128)],
                                     G_s[:, tcp, :],
                                     start=(tcp == 0), stop=(tcp == TK - 1))
                nc.vector.tensor_copy(xgT[:, ct, :], ps)
            for pt in range(CK if do_padqkv else 0):
                ps = pg.tile([128, J], F32)
                for j in range(CK):
                    nc.tensor.matmul(ps, wqk_s[:, j, ts(pt, 128)],
                                     xgT[:, j, :],
                                     start=(j == 0), stop=(j == CK - 1))
                nc.vector.tensor_scalar_add(qpT[:, pt, :], ps,
                                            bqk_s[:, pt:pt + 1])

    # ---- phase B+C: qk^T & v projections, pad-row QK/exp/AV interleaved ----
    with tc.tile_pool(name="pqk", bufs=3, space="PSUM") as pqk, \
         tc.tile_pool(name="pspad", bufs=3, space="PSUM") as pspad, \
         tc.tile_pool(name="etpad", bufs=14) as etpad, \
         tc.tile_pool(name="ypadp", bufs=1, space="PSUM") as ypadp:

        def qk_tile(i):
            for n in range(T // 512):
                ps = pqk.tile([128, 512], F32)
                for j in range(CK):
                    nc.tensor.matmul(ps, wqk_s[:, j, ts(i, 128)],
                                     xT_s[:, j, ts(n, 512)],
                                     start=(j == 0), stop=(j == CK - 1))
                nc.vector.tensor_scalar_add(qkT[:, i, ts(n, 512)], ps,
                                            bqk_s[:, i:i + 1])

        def v_tile(i):
            ps = pqk.tile([128, 512], F32)
            for j in range(CK):
                nc.tensor.matmul(ps, xT_s[:, j, ts(i, 128)], wv_s[:, j, :],
                                 start=(j == 0), stop=(j == CK - 1))
            nc.scalar.activation(
                vaug[:, i, :, 0:D], ps.rearrange("p (h d) -> p h d", h=H),
                AF.Copy)

        # k head-tiles first so the pad pass (and its exp work) starts early
        for i in range(4, 8):
            qk_tile(i)

        # pad QK/exp strips, interleaved with q/v tiles as PE filler while
        # ACT chews the pad exps; pad AV runs after v is available.
        eps = {}
        filler = [(qk_tile, 0), (qk_tile, 1), (v_tile, 0), (v_tile, 1),
                  (qk_tile, 2), (qk_tile, 3), (v_tile, 2), (v_tile, 3),
                  (v_tile, 4), (v_tile, 5), (v_tile, 6), (v_tile, 7)]
        fi = 0
        if do_strips:
            # strips grouped by head parity: 64-row matmuls must keep a
            # fixed lhsT partition-base within a strip (switching the PE
            # row-quadrant between nearby 64-row matmuls aborts the NEFF),
            # so group pg covers heads h = 2*hh + pg (all at base pg*64)
            for pg in range(2):
                po = pg * 64
                for kt in range(1, TK):
                    sp = pspad.tile([128, 4, J], F32)
                    for hh in range(4):
                        nc.tensor.matmul(sp[:, hh, :],
                                         qkT[po:po + D, 4 + hh, ts(kt, 128)],
                                         qpT[po:po + D, hh, :],
                                         start=(hh == 0), stop=(hh == 3),
                                         skip_group_check=True)
                    ep = etpad.tile([128, 4, J], BF16)
                    if strip_exp:
                        nc.scalar.activation(ep, sp, AF.Exp)
                        if strip_padm:
                            for hh in range(4):
                                nc.vector.tensor_mul(ep[:, hh, :],
                                                     ep[:, hh, :],
                                                     padm_s[:, kt, :])
                    eps[(kt, pg)] = ep
                    if strip_interleave and fi < len(filler):
                        f, arg = filler[fi]
                        f(arg)
                        fi += 1
        while fi < len(filler):
            f, arg = filler[fi]
            f(arg)
            fi += 1

        if do_padav:
            # one bank-aligned PSUM tile for both pad accumulator halves;
            # a K=1 zeros-matmul opens each half's 2KB zero-region exactly
            # once (start=True zeroes the whole bank, so per-slice starts
            # would clobber sibling head slices)
            ypp = ypadp.tile([J, 2, 512], F32)
            for pg in range(2):
                nc.tensor.matmul(ypp[:, pg, :], zcol, zrow,
                                 start=True, stop=False)
            for pg in range(2):
                for kt in range(1, TK):
                    ep = eps[(kt, pg)]
                    for hh in range(4):
                        h = 2 * hh + pg
                        nc.tensor.matmul(ypp[:, pg, ts(hh, D + 1)],
                                         ep[:, hh, :], vaug[:, kt, h, :],
                                         start=False,
                                         stop=(kt == TK - 1 and hh == 3))

            if do_ypadcopy:
                nc.vector.tensor_copy(ypad[:, 0, :], ypp[:, 0, 0:4 * (D + 1)])
                nc.vector.tensor_copy(ypad[:, 1, :], ypp[:, 1, 0:4 * (D + 1)])

    # ---- phase D: causal attention, head-outer, k-tile strips ----
    if not do_attn:
        return
    with tc.tile_pool(name="ps_s", bufs=2, space="PSUM") as ps_s, \
         tc.tile_pool(name="ps_y", bufs=2, space="PSUM") as ps_y, \
         tc.tile_pool(name="expp", bufs=9) as expp, \
         tc.tile_pool(name="rp", bufs=2) as rp, \
         tc.tile_pool(name="rbp", bufs=2) as rbp:
        for h in range(H):
            hp, po = h // 2, (h % 2) * 64
            half, hh = h % 2, h // 2
            # all QK strips + exp first: PE streams QK while ACT exps lag,
            # then the AVs (which need et) run with everything ready
            ets = []
            for kt in range(TK):
                q0 = 128 * kt
                w = T - q0
                sp = ps_s.tile([128, T], F32)
                for off, n in _chunks(w):
                    nc.tensor.matmul(sp[:, off:off + n],
                                     qkT[po:po + D, 4 + hp, ts(kt, 128)],
                                     qkT[po:po + D, hp, q0 + off:q0 + off + n],
                                     start=True, stop=True)
                et = expp.tile([128, T], BF16)
                nc.scalar.activation(et[:, 0:w], sp[:, 0:w], AF.Exp)
                # first 128 strip-columns = the diagonal block -> mask it
                nc.vector.tensor_mul(et[:, 0:128], et[:, 0:128],
                                     dmask_s[:, kt, :])
                ets.append(et)
            yp = ps_y.tile([D + 1, T], F32)
            for kt in range(TK):
                q0 = 128 * kt
                # AV output chunks aligned to absolute 512-col PSUM banks
                a = q0
                for bnd in (512, T):
                    if a < bnd:
                        n = bnd - a
                        nc.tensor.matmul(yp[:, a:a + n],
                                         vaug[:, kt, h, :],
                                         ets[kt][:, a - q0:a - q0 + n],
                                         start=(kt == 0), stop=False,
                                         skip_group_check=True)
                        a = bnd
            # scatter pad-row num/den contributions into this head's PSUM
            if do_scatter:
                for off, n in _chunks(T):
                    nc.tensor.matmul(yp[:, off:off + n],
                                     ypad[:, half, ts(hh, D + 1)],
                                     GT_s[:, off:off + n],
                                     start=False, stop=True,
                                     skip_group_check=True)
            rec = rp.tile([1, T], F32)
            nc.vector.reciprocal(rec, yp[D:D + 1, :])
            rb = rbp.tile([D, T], F32)
            nc.gpsimd.partition_broadcast(rb, rec)
            nc.vector.tensor_mul(yT[po:po + D, hp, :], yp[0:D, :], rb)

    # ---- phase E: out^T = W_p^T y^T + b_eff, store transposed ----
    if not do_proj:
        return
    with tc.tile_pool(name="pp", bufs=2, space="PSUM") as pp, \
         tc.tile_pool(name="outst", bufs=3) as outst:
        for qt in range(TK):
            op = pp.tile([128, CK, 128], F32)
            for ct in range(CK):
                for cin in range(CK):
                    nc.tensor.matmul(op[:, ct, :],
                                     wp_s[:, cin, ts(ct, 128)],
                                     yT[:, cin, ts(qt, 128)],
                                     start=(ct == 0 and cin == 0),
                                     stop=(ct == CK - 1 and cin == CK - 1),
                                     skip_group_check=True)
            ot = outst.tile([128, CK, 128], F32)
            for ct in range(CK):
                if ct % 2 == 0:
                    nc.scalar.activation(ot[:, ct, :], op[:, ct, :],
                                         AF.Identity,
                                         bias=beff_s[:, ct:ct + 1])
                else:
                    nc.vector.tensor_scalar_add(ot[:, ct, :], op[:, ct, :],
                                                beff_s[:, ct:ct + 1])
            nc.sync.dma_start(out=out_d[:, :, ts(qt, 128)], in_=ot)


def get_nc(reps=1, upto=9):
    key = ("nc", reps, upto)
    if key not in _CACHE:
        _CACHE[key] = _build_nc(reps, upto)
    return _CACHE[key]


def make_in_maps(x, padding_mask, W_qkv, b_qkv, W_proj, b_proj):
    BF = ml_dtypes.bfloat16
    x = np.asarray(x, np.float32)
    padding_mask = np.asarray(padding_mask, bool)
    W_qkv = np.asarray(W_qkv, np.float32)
    b_qkv = np.asarray(b_qkv, np.float32)
    W_proj = np.asarray(W_proj, np.float32)
    b_proj = np.asarray(b_proj, np.float32)

    scale = np.float32(1.0 / np.sqrt(D))
    wqk = np.concatenate([W_qkv[:, :C] * scale, W_qkv[:, C:2 * C]], axis=1)
    wqk = np.ascontiguousarray(
        wqk.reshape(CK, 128, 2 * C).transpose(1, 0, 2)).astype(BF)
    wv = np.ascontiguousarray(
        W_qkv[:, 2 * C:].reshape(CK, 128, C).transpose(1, 0, 2)).astype(BF)
    wp = np.ascontiguousarray(
        W_proj.reshape(CK, 128, C).transpose(1, 0, 2)).astype(BF)
    bqk = np.concatenate([b_qkv[:C] * scale, b_qkv[C:2 * C]])
    bqk = np.ascontiguousarray(bqk.reshape(-1, 128).T).astype(np.float32)
    beff = (b_qkv[2 * C:] @ W_proj + b_proj)
    beff = np.ascontiguousarray(beff.reshape(CK, 128).T).astype(np.float32)

    pp = np.arange(128)
    in_maps = []
    for b in range(B):
        tt = b % TT
        pad = padding_mask[tt]
        idx = np.where(pad)[0]
        npad = len(idx)
        assert npad <= J, f"npad={npad} exceeds gather capacity J={J}"

        G = np.zeros((T, J), np.float32)
        G[idx, np.arange(npad)] = 1.0
        G = np.ascontiguousarray(
            G.reshape(TK, 128, J).transpose(1, 0, 2)).astype(BF)
        GT = np.zeros((J, T), np.float32)
        GT[np.arange(npad), idx] = 1.0
        GT = GT.astype(BF)
        padm = np.zeros((TK, J), np.float32)
        qt_of = idx // 128
        for jj in range(npad):
            padm[qt_of[jj] + 1:, jj] = 1.0
        padm = np.ascontiguousarray(
            np.broadcast_to(padm[None], (128, TK, J))).astype(BF)
        # dmask[p, kt, q] = (p <= q) | pad[128*kt + q]
        dmask = (pp[:, None, None] <= pp[None, None, :]) | \
            pad.reshape(TK, 128)[None, :, :]
        dmask = np.ascontiguousarray(dmask).astype(BF)

        xb_full = x[b]
        xT = np.ascontiguousarray(
            xb_full.T.reshape(CK, 128, T).transpose(1, 0, 2)).astype(BF)
        xb = np.ascontiguousarray(
            xb_full.reshape(TK, 128, C).transpose(1, 0, 2)).astype(BF)

        in_maps.append({
            "xT": xT, "xb": xb, "wqk": wqk, "wv": wv, "wp": wp,
            "bqk": bqk, "beff": beff, "dmask": dmask,
            "G": G, "GT": GT, "padm": padm,
        })
    return in_maps


def kernel(x, padding_mask, W_qkv, b_qkv, W_proj, b_proj):
    from concourse.bass_utils import run_bass_kernel_spmd

    nc = get_nc()
    in_maps = make_in_maps(x, padding_mask, W_qkv, b_qkv, W_proj, b_proj)
    res = run_bass_kernel_spmd(nc, in_maps, list(range(NCORES)))
    outs = []
    for b in range(B):
        a = res.results[b]["out"]          # [128, CK, T] = out^T tiled
        outT = a.transpose(1, 0, 2).reshape(C, T)
        outs.append(outT.T)
    return np.ascontiguousarray(np.stack(outs)).astype(np.float32)
